# revision 6
# baseline (speedup 1.0000x reference)
"""Trainium2 Bass kernel for MetaLayer-style GNN (edge/node/global GRU message passing).

Contract: kernel(**inputs) takes the FULL unsharded inputs (np arrays, keys as in
setup_inputs) and returns the FULL output [B, STEPS, H] float32.

Strategy (8 NeuronCores):
- Sort edges by dst, shard nodes into 8 equal contiguous ranges; each core owns all
  edges whose dst is in its range => node aggregation is core-local.
- Per step: edge MLP+GRU (edge-parallel, bf16 matmuls, col-form activations),
  windowed one-hot matmul aggregation, node MLP+GRU on local nodes, AllGather of
  updated x (bf16) to rebuild the replicated gather table, small AllReduce for
  per-graph node means, replicated global MLP+GRU on every core.
- x[src] rows fetched with hardware-dynamic indirect DMA (128 rows/call) straight
  from the replicated x_full table; DMA-XBAR transposes (dma_start_transpose)
  convert row-form gathers / chunks between [H, e] and [e, H] forms with zero
  TensorE cost.
- All one-hot matrices (dst-window expansion D, aggregation A with 1/cnt folded,
  u[batch[src]] selector SU, node-phase S_nb / Bmat) are generated ON DEVICE from
  tiny index columns via iota + tensor_scalar(is_equal), instead of streaming
  dense host-built blobs from HBM.
- MLP second layer folded into GRU input weights: gi = relu_h1 @ (Wih@W2).T + ...
- GRU elementwise tail in bf16 for DVE 2x mode.
"""

from contextlib import ExitStack

import numpy as np
import ml_dtypes

import concourse.bass as bass
import concourse.bacc as bacc
import concourse.tile as tile
from concourse import mybir
from concourse.bass import IndirectOffsetOnAxis
from concourse.bass_utils import run_bass_kernel_spmd
from concourse.masks import make_identity

BF16 = ml_dtypes.bfloat16
AF = mybir.ActivationFunctionType
DT = mybir.dt
ALU = mybir.AluOpType

# ---------------------------------------------------------------- configuration

class Cfg:
    def __init__(self, N=50000, E=500000, B=64, H=128, STEPS=3, NCORES=8, CH=512):
        assert H == 128
        assert N % NCORES == 0
        self.N, self.E, self.B, self.H, self.STEPS, self.NCORES = N, E, B, H, STEPS, NCORES
        self.CH = CH                      # edge chunk (free dim of f32 PSUM <= 512)
        self.NL = N // NCORES             # local nodes
        self.NLP = ((self.NL + CH - 1) // CH) * CH
        self.NCHN = self.NLP // CH        # node chunks
        self.GRP = 4                      # chunks per eT staging / agg group

    def finalize(self, max_shard_edges):
        gran = self.CH * self.GRP
        self.EPAD = ((max_shard_edges + gran - 1) // gran) * gran
        self.NCHE = self.EPAD // self.CH  # edge chunks
        self.NSUBS = self.EPAD // 128     # 128-edge subs
        self.AW = 256                     # aggregation window width (nodes)
        # data-independent window start per sub (aligned 128, clamped)
        self.wstart = []
        for sub in range(self.NSUBS):
            c = (sub + 0.5) * 128 * self.NL / self.EPAD
            w = 128 * int(c // 128) - 64
            w = max(0, min(w, self.NLP - self.AW))
            self.wstart.append(w)
        # chunk-level windows for the x[dst] expansion matmuls (128-aligned)
        self.w2start = []
        for k in range(self.NCHE):
            c = (k + 0.5) * self.CH * self.NL / self.EPAD
            w = 128 * int((c - 64) // 128)
            w = max(0, min(w, self.NLP - self.AW))
            self.w2start.append(w)
        self.NBLK = self.NLP // 128       # PXrow blocks
        # consts tile layout (bf16 cols)
        self.C_IOTA = 0                   # 256 cols: value = col index
        self.C_B64 = 256                  # 64 cols: value = col index (graph ids)
        self.C_W = 320
        return self


# ---------------------------------------------------------------- host helpers

def host_prepare(cfg, inputs):
    N, E, B, H = cfg.N, cfg.E, cfg.B, cfg.H
    x = np.asarray(inputs['x'], np.float32)
    edge_index = np.asarray(inputs['edge_index'])
    edge_attr = np.asarray(inputs['edge_attr'], np.float32)
    u = np.asarray(inputs['u'], np.float32)
    batch = np.asarray(inputs['batch']).astype(np.int64)
    src, dst = edge_index[0].astype(np.int64), edge_index[1].astype(np.int64)

    def g(name):
        return np.asarray(inputs[name], np.float32)

    W1, b1 = g('edge_w1'), g('edge_b1')
    W2, b2 = g('edge_w2'), g('edge_b2')
    eWih, eWhh = g('egru_wih'), g('egru_whh')
    eBih, eBhh = g('egru_bih'), g('egru_bhh')
    nW1, nb1 = g('node_w1'), g('node_b1')
    nW2, nb2 = g('node_w2'), g('node_b2')
    nWih, nWhh = g('ngru_wih'), g('ngru_whh')
    nBih, nBhh = g('ngru_bih'), g('ngru_bhh')
    gW1, gb1 = g('glob_w1'), g('glob_b1')
    gW2, gb2 = g('glob_w2'), g('glob_b2')
    gWih, gWhh = g('ggru_wih'), g('ggru_whh')
    gBih, gBhh = g('ggru_bih'), g('ggru_bhh')

    eWih2, eBih2 = eWih @ W2, eWih @ b2 + eBih
    nWih2, nBih2 = nWih @ nW2, nWih @ nb2 + nBih
    gWih2, gBih2 = gWih @ gW2, gWih @ gb2 + gBih

    def gate(Wm, i):
        return Wm[i * H:(i + 1) * H, :].T

    blocks = [
        W1[:, 0:H].T, W1[:, H:2 * H].T, W1[:, 2 * H:3 * H].T, W1[:, 3 * H:4 * H].T,
        gate(eWih2, 0), gate(eWih2, 1), gate(eWih2, 2),
        gate(eWhh, 0), gate(eWhh, 1), gate(eWhh, 2),
        nW1[:, 0:H].T, nW1[:, H:2 * H].T, nW1[:, 2 * H:3 * H].T,
        gate(nWih2, 0), gate(nWih2, 1), gate(nWih2, 2),
        gate(nWhh, 0), gate(nWhh, 1), gate(nWhh, 2),
        gW1[:, 0:H].T, gW1[:, H:2 * H].T,
        gate(gWih2, 0), gate(gWih2, 1), gate(gWih2, 2),
        gate(gWhh, 0), gate(gWhh, 1), gate(gWhh, 2),
    ]
    wpk = np.concatenate([bl.astype(np.float32) for bl in blocks], axis=1).astype(BF16)

    def gb_(v, i):
        return v[i * H:(i + 1) * H]

    bcols = [
        b1, gb_(eBih2, 0) + gb_(eBhh, 0), gb_(eBih2, 1) + gb_(eBhh, 1), gb_(eBhh, 2), gb_(eBih2, 2),
        nb1, gb_(nBih2, 0) + gb_(nBhh, 0), gb_(nBih2, 1) + gb_(nBhh, 1), gb_(nBhh, 2), gb_(nBih2, 2),
        gb1, gb_(gBih2, 0) + gb_(gBhh, 0), gb_(gBih2, 1) + gb_(gBhh, 1), gb_(gBhh, 2), gb_(gBih2, 2),
    ]
    bpk = np.stack(bcols, axis=1).astype(np.float32)

    order = np.argsort(dst, kind='stable')
    ssrc, sdst, sea = src[order], dst[order], edge_attr[order]
    shard_of = sdst // cfg.NL
    counts = np.bincount(shard_of, minlength=cfg.NCORES)
    cfg.finalize(int(counts.max()))

    gcnt = np.bincount(batch, minlength=B).astype(np.float32)
    ginv = 1.0 / np.maximum(gcnt, 1.0)
    ncnt = np.bincount(sdst, minlength=N).astype(np.float32)
    ninv = 1.0 / np.maximum(ncnt, 1.0)
    bsrc_all = batch[ssrc]

    # shared constants
    consts = np.zeros((128, cfg.C_W), np.float32)
    consts[:, cfg.C_IOTA:cfg.C_IOTA + 256] = np.arange(256)[None, :]
    consts[:, cfg.C_B64:cfg.C_B64 + 64] = np.arange(64)[None, :]
    consts = consts.astype(BF16)
    colf = np.zeros((128, 2), np.float32)
    colf[:, 0] = np.arange(128)
    colf[:, 1] = np.arange(128) + 128
    ginvb = np.ascontiguousarray(np.broadcast_to(ginv[None, :], (128, B))).astype(np.float32)

    xb = x.astype(BF16)
    in_maps = []
    bounds = np.searchsorted(sdst, np.arange(cfg.NCORES + 1) * cfg.NL)
    for c in range(cfg.NCORES):
        lo_, hi_ = int(bounds[c]), int(bounds[c + 1])
        ne = hi_ - lo_
        npad = cfg.EPAD - ne
        base = c * cfg.NL
        nl, nlp = cfg.NL, cfg.NLP

        # Interleave pads uniformly so slot->node quantile mapping matches the
        # program-uniform window schedule (all-at-end padding would drift).
        pad_slots = np.unique(np.round(np.linspace(0, cfg.EPAD - 1, npad)).astype(np.int64)) \
            if npad > 0 else np.empty(0, np.int64)
        while pad_slots.shape[0] < npad:
            extra = np.setdiff1d(np.arange(cfg.EPAD), pad_slots)[:npad - pad_slots.shape[0]]
            pad_slots = np.union1d(pad_slots, extra)
        is_pad = np.zeros(cfg.EPAD, bool)
        is_pad[pad_slots] = True
        eslot = np.nonzero(~is_pad)[0]                     # slot of real edge i

        def scatter_edges(vals, padval, dtype=np.float32):
            out = np.full(cfg.EPAD, padval, dtype)
            out[eslot] = vals
            return out

        csrc = ssrc[lo_:hi_]
        cdst_loc = sdst[lo_:hi_] - base
        cbsrc = bsrc_all[lo_:hi_]

        # src gather index table: idx0[p, s] = src at slot s*128+p
        src_slot = scatter_edges(csrc, 0, np.int64)
        idx0 = np.ascontiguousarray(
            src_slot.reshape(cfg.NSUBS, 128).T.astype(np.int32))

        # per-chunk rows: rel2 (dst window offset) and bsrc along the free dim
        w2 = np.asarray(cfg.w2start)                       # [NCHE]
        rel2_e = cdst_loc - w2[eslot // cfg.CH]
        assert rel2_e.min() >= 0 and rel2_e.max() < cfg.AW, \
            f"dst window violated: {rel2_e.min()} {rel2_e.max()}"
        rel2 = scatter_edges(rel2_e, -1.0)
        bsrc_s = scatter_edges(cbsrc, -1.0)
        rows2 = np.concatenate([rel2.reshape(cfg.NCHE, 1, cfg.CH),
                                bsrc_s.reshape(cfg.NCHE, 1, cfg.CH)],
                               axis=2).astype(BF16)

        # per-chunk agen: [128, 8] = relA cols (4 subs) + ninv cols (4 subs)
        ws = np.asarray(cfg.wstart)                        # [NSUBS]
        relA_e = cdst_loc - ws[eslot // 128]
        assert relA_e.min() >= 0 and relA_e.max() < cfg.AW, \
            f"agg window violated: {relA_e.min()} {relA_e.max()}"
        relA = scatter_edges(relA_e, -1.0)
        ninv_loc = ninv[base:base + nl]
        ninvA = scatter_edges(ninv_loc[cdst_loc], 0.0)
        agen = np.zeros((cfg.NCHE, 128, 8), np.float32)
        agen[:, :, 0:4] = relA.reshape(cfg.NCHE, 4, 128).transpose(0, 2, 1)
        agen[:, :, 4:8] = ninvA.reshape(cfg.NCHE, 4, 128).transpose(0, 2, 1)
        agen = agen.astype(np.float32)

        # node-phase: batch ids per local node (padded with -1)
        batch_loc = batch[base:base + nl].astype(np.float32)
        bl_pad = np.concatenate([batch_loc, np.full(nlp - nl, -1.0, np.float32)])
        nagen = np.ascontiguousarray(
            bl_pad.reshape(cfg.NCHN, 4, 128).transpose(0, 2, 1)).astype(np.float32)
        nrows = np.ascontiguousarray(
            bl_pad.reshape(cfg.NCHN, 1, cfg.CH)).astype(BF16)

        xT0 = np.zeros((128, nlp), np.float32)
        xT0[:, :nl] = x[base:base + nl].T
        eT0 = np.zeros((128, cfg.EPAD), BF16)
        eT0[:, eslot] = sea[lo_:hi_].T.astype(BF16)

        in_maps.append(dict(
            wpk=wpk, bpk=bpk,
            xT0=xT0,
            uT0=np.ascontiguousarray(u.T).astype(np.float32),
            eT0=eT0,
            x0b=xb,
            idx0=idx0,
            agen=agen,
            rows2=rows2,
            nagen=nagen,
            nrows=nrows,
            consts=consts,
            colf=colf,
            ginvb=ginvb,
        ))
    return in_maps


# ---------------------------------------------------------------- device program

def build_program(cfg):
    nc = bacc.Bacc("TRN2", target_bir_lowering=False, debug=False,
                   num_devices=cfg.NCORES, num_swdge_queues=2)
    H, B, CH = cfg.H, cfg.B, cfg.CH
    NW = 27
    f32, bf16, i32 = DT.float32, DT.bfloat16, DT.int32

    def din(name, shape, dt):
        return nc.dram_tensor(name, shape, dt, kind="ExternalInput").ap()

    t = {}
    t['wpk'] = din("wpk", [128, NW * 128], bf16)
    t['bpk'] = din("bpk", [128, 15], f32)
    t['xT0'] = din("xT0", [128, cfg.NLP], f32)
    t['uT0'] = din("uT0", [128, B], f32)
    t['eT0'] = din("eT0", [128, cfg.EPAD], bf16)
    t['x0b'] = din("x0b", [cfg.N, H], bf16)
    t['idx0'] = din("idx0", [128, cfg.NSUBS], i32)
    t['agen'] = din("agen", [cfg.NCHE, 128, 8], f32)
    t['rows2'] = din("rows2", [cfg.NCHE, 1, 2 * CH], bf16)
    t['nagen'] = din("nagen", [cfg.NCHN, 128, 4], f32)
    t['nrows'] = din("nrows", [cfg.NCHN, 1, CH], bf16)
    t['consts'] = din("consts", [128, cfg.C_W], bf16)
    t['colf'] = din("colf", [128, 2], f32)
    t['ginvb'] = din("ginvb", [128, B], f32)

    t['out'] = nc.dram_tensor("out", [B, cfg.STEPS, H], f32, kind="ExternalOutput").ap()

    t['eTd'] = [nc.dram_tensor(f"eTd{i}", [128, cfg.EPAD], bf16).ap() for i in range(2)]
    t['x_shard'] = nc.dram_tensor("x_shard", [cfg.NL, H], bf16).ap()
    t['x_full'] = nc.dram_tensor("x_full", [cfg.N, H], bf16, addr_space="Shared").ap()
    t['gsum_in'] = nc.dram_tensor("gsum_in", [128, B], f32).ap()
    t['gsum_out'] = nc.dram_tensor("gsum_out", [128, B], f32, addr_space="Shared").ap()
    t['rg'] = [list(range(cfg.NCORES))]

    with ExitStack() as ctx:
        tc = ctx.enter_context(tile.TileContext(nc))
        _emit(nc, tc, ctx, cfg, t)
    nc.compile()
    return nc


def _emit(nc, tc, ctx, cfg, t):
    H, B, CH = cfg.H, cfg.B, cfg.CH
    f32, bf16, i32 = DT.float32, DT.bfloat16, DT.int32
    NSUB = CH // 128
    GRP = cfg.GRP

    perm = ctx.enter_context(tc.tile_pool(name="perm", bufs=1))
    sb = ctx.enter_context(tc.tile_pool(name="sb", bufs=3))
    sb2 = ctx.enter_context(tc.tile_pool(name="sb2", bufs=2))
    ps_h1 = ctx.enter_context(tc.tile_pool(name="ps_h1", bufs=2, space="PSUM"))
    ps_g = ctx.enter_context(tc.tile_pool(name="ps_g", bufs=1, space="PSUM"))
    ps_tp = ctx.enter_context(tc.tile_pool(name="ps_tp", bufs=1, space="PSUM"))

    # ---------------- persistent SBUF state
    W = perm.tile([128, 27 * 128], bf16)
    nc.sync.dma_start(W[:], t['wpk'][:])

    def w(i):
        return W[:, i * 128:(i + 1) * 128]

    bias = perm.tile([128, 15], f32)
    nc.sync.dma_start(bias[:], t['bpk'][:])

    def bv(i):
        return bias[:, i:i + 1]

    xT = perm.tile([128, cfg.NLP], f32)
    nc.sync.dma_start(xT[:], t['xT0'][:])
    xTb = perm.tile([128, cfg.NLP], bf16)
    nc.vector.tensor_copy(xTb[:], xT[:])

    uT = perm.tile([128, B], f32)
    nc.sync.dma_start(uT[:], t['uT0'][:])
    uTb = perm.tile([128, B], bf16)
    nc.vector.tensor_copy(uTb[:], uT[:])

    bsum_acc = perm.tile([128, B], f32)
    aggT = perm.tile([128, cfg.NLP], bf16)    # resident aggregation accumulator
    # W1b-projected x rows, 128-aligned blocks (for the x[dst] expansion)
    PXa = perm.tile([128, cfg.NBLK, 128], bf16)

    ident_f = perm.tile([128, 128], f32)
    make_identity(nc, ident_f[:])

    idxT = perm.tile([128, cfg.NSUBS], i32)   # src node id per slot
    nc.sync.dma_start(idxT[:], t['idx0'][:])

    consts = perm.tile([128, cfg.C_W], bf16)
    nc.sync.dma_start(consts[:], t['consts'][:])
    iota256 = consts[:, cfg.C_IOTA:cfg.C_IOTA + 256]
    iotaB = consts[:, cfg.C_B64:cfg.C_B64 + B]
    colf = perm.tile([128, 2], f32)
    nc.sync.dma_start(colf[:], t['colf'][:])
    iotaCol = colf[:, 0:1]
    iotaCol1 = colf[:, 1:2]

    ginvb = perm.tile([128, B], f32)
    nc.sync.dma_start(ginvb[:], t['ginvb'][:])

    # ---------------- init DRAM state
    nc.sync.dma_start(t['eTd'][0][:], t['eT0'][:])
    nc.sync.dma_start(t['x_full'][:], t['x0b'][:])

    def gru_mm(xiT, hTb, wb, FD):
        """GRU gate matmuls: returns (pr, pz, pig, phg) PSUM tiles."""
        pr = ps_g.tile([128, FD], f32, tag="pr")
        nc.tensor.matmul(pr[:], lhsT=w(wb + 0), rhs=xiT, start=True, stop=False)
        nc.tensor.matmul(pr[:], lhsT=w(wb + 3), rhs=hTb, start=False, stop=True)
        pz = ps_g.tile([128, FD], f32, tag="pz")
        nc.tensor.matmul(pz[:], lhsT=w(wb + 1), rhs=xiT, start=True, stop=False)
        nc.tensor.matmul(pz[:], lhsT=w(wb + 4), rhs=hTb, start=False, stop=True)
        pig = ps_g.tile([128, FD], f32, tag="pig")
        nc.tensor.matmul(pig[:], lhsT=w(wb + 2), rhs=xiT, start=True, stop=True)
        phg = ps_g.tile([128, FD], f32, tag="phg")
        nc.tensor.matmul(phg[:], lhsT=w(wb + 5), rhs=hTb, start=True, stop=True)
        return pr, pz, pig, phg

    def gru_tail(ps4, hTb, bb, pool, h_f32, out_ap, FD):
        """GRU elementwise tail: acts + DVE blend. If h_f32 given: blend in f32
        in-place there; else write bf16 into out_ap."""
        pr, pz, pig, phg = ps4
        r = pool.tile([128, FD], bf16, tag="r", bufs=2)
        nc.scalar.activation(r[:], pr[:], AF.Sigmoid, bias=bv(bb + 0))
        z = pool.tile([128, FD], bf16, tag="z", bufs=2)
        nc.scalar.activation(z[:], pz[:], AF.Sigmoid, bias=bv(bb + 1))
        hg = pool.tile([128, FD], bf16, tag="hg", bufs=2)
        nc.scalar.activation(hg[:], phg[:], AF.Identity, bias=bv(bb + 2))
        tm = pool.tile([128, FD], bf16, tag="tm")
        nc.vector.tensor_tensor(tm[:], r[:], hg[:], op=ALU.mult)
        sp = pool.tile([128, FD], bf16, tag="sp")
        nc.vector.tensor_tensor(sp[:], tm[:], pig[:], op=ALU.add)
        n = pool.tile([128, FD], bf16, tag="n")
        nc.scalar.activation(n[:], sp[:], AF.Tanh, bias=bv(bb + 3))

        d = pool.tile([128, FD], bf16, tag="d")
        nc.vector.tensor_tensor(d[:], hTb, n[:], op=ALU.subtract)
        m = pool.tile([128, FD], bf16, tag="m")
        nc.vector.tensor_tensor(m[:], z[:], d[:], op=ALU.mult)
        if h_f32 is not None:
            nc.vector.tensor_tensor(h_f32, n[:], m[:], op=ALU.add)
        else:
            nc.vector.tensor_tensor(out_ap, n[:], m[:], op=ALU.add)

    def gru(xiT, hTb, wb, bb, pool, h_f32, out_ap, FD):
        gru_tail(gru_mm(xiT, hTb, wb, FD), hTb, bb, pool, h_f32, out_ap, FD)

    def gather_chunk(k):
        """Issue 4 indirect-DMA row gathers for chunk k -> g_row [128, 4, 128]."""
        gr = sb.tile([128, NSUB, 128], bf16, tag="g_row", bufs=4, name="g_row")
        for j in range(NSUB):
            col = k * NSUB + j
            nc.gpsimd.indirect_dma_start(
                out=gr[:, j, :],
                out_offset=None,
                in_=t['x_full'][:],
                in_offset=IndirectOffsetOnAxis(ap=idxT[:, col:col + 1], axis=0),
            )
        return gr

    for s in range(cfg.STEPS):
        eT_r, eT_w = t['eTd'][s % 2], t['eTd'][(s + 1) % 2]
        nc.vector.memset(aggT[:], 0.0)

        # per-step u projections: uWd_row = u @ W1d.T ; uWnc_row = u @ Wn1c.T
        uprj = []
        for wi, tg in ((3, "uprj_e"), (12, "uprj_n")):
            p = ps_g.tile([B, 128], f32, tag="pr")
            nc.tensor.matmul(p[:], lhsT=uTb[:], rhs=w(wi), start=True, stop=True)
            srow = sb2.tile([B, 128], bf16, tag=tg)
            nc.vector.tensor_copy(srow[:], p[:])
            uprj.append(srow)
        uWd_row, uWnc_row = uprj

        # PXrow: per 128-node block, rows of x @ W1b.T
        for blk in range(cfg.NBLK):
            base = blk * 128
            px = ps_h1.tile([128, 128], f32, tag="h1")
            nc.tensor.matmul(px[:], lhsT=xTb[:, base:base + 128],
                             rhs=w(1), start=True, stop=True)
            nc.vector.tensor_copy(PXa[:, blk, :], px[:])

        # ================= EDGE PHASE (software-pipelined emission) ==========
        # iteration k emits: gather k+2; blob DMA k+1; one-hot gen + xbar + h1
        # for chunk k; relu+GRU for chunk k-1; at group boundaries: store +
        # row-transpose/aggregate the PREVIOUS group.
        st = {}                           # per-chunk saved refs
        gtile = {}                        # group -> eT_out tile
        grow = {}                         # chunk -> gathered src rows
        blobs = {}                        # chunk -> (agen tile, rel2b, bsrcb)

        def load_blobs(k):
            ag = sb.tile([128, 8], f32, tag="agen", bufs=GRP + 3, name="agen")
            nc.sync.dma_start(ag[:], t['agen'][k, :, :])
            rw = sb.tile([1, 2 * CH], bf16, tag="rows2", bufs=2, name="rows2")
            nc.sync.dma_start(rw[:], t['rows2'][k, :, :])
            rel2b = sb.tile([128, CH], bf16, tag="rel2b", bufs=2)
            nc.gpsimd.partition_broadcast(rel2b[:], rw[0:1, 0:CH])
            bsrcb = sb.tile([128, CH], bf16, tag="bsrcb", bufs=2)
            nc.gpsimd.partition_broadcast(bsrcb[:], rw[0:1, CH:2 * CH])
            blobs[k] = (ag, rel2b, bsrcb)

        def agg_block(g):
            """Row-transpose (XBAR) + one-hot aggregate all chunks of group g."""
            out_t = gtile.pop(g)
            for ci in range(GRP):
                k_ = g * GRP + ci
                ag = st.pop(('ag', k_))
                erow = sb.tile([128, NSUB, 128], bf16, tag="erow", bufs=2)
                nc.scalar.dma_start(erow[:], out_t[:, ci * CH:(ci + 1) * CH],
                                    transpose=True)
                for j in range(NSUB):
                    # A tile: one-hot [e, win] with 1/cnt folded, on the fly
                    at = sb.tile([128, cfg.AW], bf16, tag="atile", bufs=3)
                    nc.vector.tensor_scalar(
                        at[:], iota256, ag[:, j:j + 1], ag[:, 4 + j:5 + j],
                        op0=ALU.is_equal, op1=ALU.mult)
                    gs = k_ * NSUB + j
                    wb = cfg.wstart[gs]
                    first = (gs == 0) or (cfg.wstart[gs - 1] != wb)
                    last = (gs == cfg.NSUBS - 1) or (cfg.wstart[gs + 1] != wb)
                    if first:
                        aw_t = ps_tp.tile([128, cfg.AW], f32, tag="aw", name="aw")
                        st['aw'] = aw_t
                    nc.tensor.matmul(st['aw'][:], lhsT=erow[:, j, :],
                                     rhs=at[:], start=first, stop=last)
                    if last:
                        nc.vector.tensor_tensor(aggT[:, wb:wb + cfg.AW],
                                                aggT[:, wb:wb + cfg.AW],
                                                st['aw'][:], op=ALU.add)

        # prologue: gathers for chunks 0,1; blobs for chunk 0
        grow[0] = gather_chunk(0)
        grow[1] = gather_chunk(1)
        load_blobs(0)

        for k in range(cfg.NCHE + 1):
            if k < cfg.NCHE:
                if k + 2 < cfg.NCHE:
                    grow[k + 2] = gather_chunk(k + 2)
                if k + 1 < cfg.NCHE:
                    load_blobs(k + 1)

                if k % GRP == 0:
                    ge = slice(k * CH, (k + GRP) * CH)
                    eT_blk = sb.tile([128, GRP * CH], bf16, tag="eT_blk", bufs=2)
                    nc.sync.dma_start(eT_blk[:], eT_r[:, ge])
                    eT_out = sb.tile([128, GRP * CH], bf16, tag="eT_out", bufs=2)
                    gtile[k // GRP] = eT_out
                    st[('blk', k // GRP)] = eT_blk

                ag, rel2b, bsrcb = blobs.pop(k)
                st[('ag', k)] = ag

                # one-hot generation for chunk k
                d0 = sb.tile([128, CH], bf16, tag="d0", bufs=2)
                nc.vector.tensor_scalar(d0[:], rel2b[:], iotaCol, None,
                                        op0=ALU.is_equal)
                d1 = sb.tile([128, CH], bf16, tag="d1", bufs=2)
                nc.vector.tensor_scalar(d1[:], rel2b[:], iotaCol1, None,
                                        op0=ALU.is_equal)
                su = sb.tile([128, CH], bf16, tag="su", bufs=2)
                nc.vector.tensor_scalar(su[:], bsrcb[:], iotaCol, None,
                                        op0=ALU.is_equal)

                # XBAR: gathered src rows [e,(j),H] -> col form [H, (j,e)]
                g_srcT = sb.tile([128, NSUB, 128], bf16, tag="g_srcT", bufs=2)
                nc.sync.dma_start(g_srcT[:], grow.pop(k)[:], transpose=True)

                koff = (k % GRP) * CH
                eT_blk = st[('blk', k // GRP)]
                eT_c = eT_blk[:, koff:koff + CH]

                w2 = cfg.w2start[k]
                assert w2 % 128 == 0
                pxh0 = PXa[:, w2 // 128, :]
                pxh1 = PXa[:, w2 // 128 + 1, :]

                h1 = ps_h1.tile([128, CH], f32, tag="h1")
                nc.tensor.matmul(h1[:], lhsT=w(0), rhs=g_srcT[:], start=True, stop=False)
                nc.tensor.matmul(h1[:], lhsT=pxh0, rhs=d0[:], start=False, stop=False)
                nc.tensor.matmul(h1[:], lhsT=pxh1, rhs=d1[:], start=False, stop=False)
                nc.tensor.matmul(h1[:], lhsT=w(2), rhs=eT_c, start=False, stop=False)
                nc.tensor.matmul(h1[:], lhsT=uWd_row[:], rhs=su[0:B, :],
                                 start=False, stop=True)
                st[k] = (h1, eT_c, koff, gtile[k // GRP])

            if k >= 1:
                h1p, eT_cp, koffp, out_tp = st.pop(k - 1)
                rh1 = sb.tile([128, CH], bf16, tag="rh1")
                nc.scalar.activation(rh1[:], h1p[:], AF.Relu, bias=bv(0))
                ps4 = gru_mm(rh1[:], eT_cp, 4, CH)
                gru_tail(ps4, eT_cp, 1, sb, None, out_tp[:, koffp:koffp + CH], CH)

            if k >= GRP and k % GRP == 0:
                g = k // GRP - 1
                nc.sync.dma_start(eT_w[:, g * GRP * CH:(g + 1) * GRP * CH],
                                  gtile[g][:])
                st.pop(('blk', g))
                agg_block(g)

        # ================= NODE PHASE (pipelined like edge phase) ============
        nst = {}
        for k in range(cfg.NCHN + 2):
            if k < cfg.NCHN:
                cn = slice(k * CH, (k + 1) * CH)
                nag = sb.tile([128, 4], f32, tag="nagen", bufs=4)
                nc.sync.dma_start(nag[:], t['nagen'][k, :, :])
                nrw = sb.tile([1, CH], bf16, tag="nrows", bufs=2)
                nc.sync.dma_start(nrw[:], t['nrows'][k, :, :])
                batchb = sb.tile([128, CH], bf16, tag="batchb", bufs=2)
                nc.gpsimd.partition_broadcast(batchb[:], nrw[0:1, :])
                snb = sb.tile([128, CH], bf16, tag="snb", bufs=2)
                nc.vector.tensor_scalar(snb[:], batchb[:], iotaCol, None,
                                        op0=ALU.is_equal)
                h1 = ps_h1.tile([128, CH], f32, tag="h1")
                nc.tensor.matmul(h1[:], lhsT=w(10), rhs=xTb[:, cn],
                                 start=True, stop=False)
                nc.tensor.matmul(h1[:], lhsT=w(11), rhs=aggT[:, cn],
                                 start=False, stop=False)
                nc.tensor.matmul(h1[:], lhsT=uWnc_row[:], rhs=snb[0:B, :],
                                 start=False, stop=True)
                nst[k] = (h1, nag, cn)

            if 1 <= k <= cfg.NCHN:
                h1p, _, cnp = nst[k - 1]
                rh1 = sb.tile([128, CH], bf16, tag="rh1")
                nc.scalar.activation(rh1[:], h1p[:], AF.Relu, bias=bv(5))
                ps4 = gru_mm(rh1[:], xTb[:, cnp], 13, CH)
                gru_tail(ps4, xTb[:, cnp], 6, sb, xT[:, cnp], None, CH)
                nc.vector.tensor_copy(xTb[:, cnp], xT[:, cnp])

            if k >= 2:
                kq = k - 2
                _, nagq, _ = nst.pop(kq)
                # row-form x via XBAR for AllGather input + graph sums
                xrow = sb.tile([128, NSUB, 128], bf16, tag="xrow", bufs=2)
                nc.scalar.dma_start(xrow[:], xTb[:, kq * CH:(kq + 1) * CH],
                                    transpose=True)
                bmm = ps_g.tile([128, B], f32, tag="aw")
                for j in range(NSUB):
                    base = kq * CH + j * 128
                    nrows_ = max(0, min(128, cfg.NL - base))
                    if nrows_ > 0 and s < cfg.STEPS - 1:
                        nc.sync.dma_start(t['x_shard'][base:base + nrows_, :],
                                          xrow[0:nrows_, j, :])
                    bmat = sb.tile([128, B], bf16, tag="bmat", bufs=2)
                    nc.vector.tensor_scalar(bmat[:], iotaB, nagq[:, j:j + 1],
                                            None, op0=ALU.is_equal)
                    nc.tensor.matmul(bmm[:], lhsT=xrow[:, j, :], rhs=bmat[:],
                                     start=(j == 0), stop=(j == NSUB - 1))
                if kq == 0:
                    nc.vector.tensor_copy(bsum_acc[:], bmm[:])
                else:
                    nc.vector.tensor_tensor(bsum_acc[:], bsum_acc[:], bmm[:],
                                            op=ALU.add)

        # ================= GLOBAL PHASE =================
        nc.sync.dma_start(t['gsum_in'][:], bsum_acc[:])
        nc.gpsimd.collective_compute(
            "AllReduce", ALU.add, replica_groups=t['rg'],
            ins=[t['gsum_in'][:]], outs=[t['gsum_out'][:]])
        # AllGather x right behind it (independent; unblocks next step)
        if s < cfg.STEPS - 1:
            nc.gpsimd.collective_compute(
                "AllGather", ALU.bypass, replica_groups=t['rg'],
                ins=[t['x_shard'][:]], outs=[t['x_full'][:]])
        nmF = sb2.tile([128, B], f32, tag="nmF")
        nc.sync.dma_start(nmF[:], t['gsum_out'][:])
        nmT = sb2.tile([128, B], bf16, tag="nmT")
        nc.vector.tensor_tensor(nmT[:], nmF[:], ginvb[:], op=ALU.mult)

        h1g = ps_h1.tile([128, B], f32, tag="h1")
        nc.tensor.matmul(h1g[:], lhsT=w(19), rhs=uTb[:], start=True, stop=False)
        nc.tensor.matmul(h1g[:], lhsT=w(20), rhs=nmT[:], start=False, stop=True)
        rh1g = sb2.tile([128, B], bf16, tag="rh1g")
        nc.scalar.activation(rh1g[:], h1g[:], AF.Relu, bias=bv(10))

        gru(rh1g[:], uTb[:], 21, 11, sb2, uT[:], None, B)
        nc.vector.tensor_copy(uTb[:], uT[:])

        utp = ps_tp.tile([B, 128], f32, tag="aw")
        nc.tensor.transpose(utp[:], uT[:], ident_f[:])
        urow = sb2.tile([B, 128], f32, tag="urow")
        nc.vector.tensor_copy(urow[:], utp[:])
        nc.sync.dma_start(t['out'][:, s, :], urow[:])


# ---------------------------------------------------------------- entry point

_CACHE = {}


def kernel(**inputs):
    x = np.asarray(inputs['x'])
    ei = np.asarray(inputs['edge_index'])
    u = np.asarray(inputs['u'])
    cfg = Cfg(N=x.shape[0], E=ei.shape[1], B=u.shape[0], H=x.shape[1], STEPS=3)
    in_maps = host_prepare(cfg, inputs)
    key = (cfg.N, cfg.E, cfg.B, cfg.H, cfg.STEPS, cfg.EPAD)
    if key not in _CACHE:
        _CACHE[key] = build_program(cfg)
    nc = _CACHE[key]
    res = run_bass_kernel_spmd(nc, in_maps, list(range(cfg.NCORES)))
    return np.asarray(res.results[0]["out"], np.float32)
